# revision 2
# baseline (speedup 1.0000x reference)
"""AGThenGemm: act = A @ W_prev (column-sharded), AllGather(act), out = act @ W_up (column-sharded).

Tensor-parallel across 8 NeuronCores:
  - W_prev sharded column-wise (D_local = D/8), W_up sharded column-wise (F_local = F/8).
  - A_prev replicated (pre-transposed on host so the contraction dim is on partitions).
  - GEMM1 emits act TRANSPOSED ([D_local, B]) so the AllGather's concat-along-first-axis
    concatenates along D, yielding act_T_global [D, B] directly usable as GEMM2's kxm.
  - Chunked over B so AllGather(chunk c) overlaps GEMM1(chunk c+1) on the TensorEngine.
"""

import numpy as np

import concourse.bass as bass
import concourse.tile as tile
from concourse import bacc, mybir
from concourse.bass_utils import run_bass_kernel_spmd
from concourse.kernels.tile_matmul import matmul_tile_kernel

N_CORES = 8
B, K_PREV, D, F = 2048, 4096, 4096, 16384
D_LOCAL = D // N_CORES
F_LOCAL = F // N_CORES
N_CHUNKS = 4


def build_nc(
    b=B,
    k_prev=K_PREV,
    d_local=D_LOCAL,
    f_local=F_LOCAL,
    n_cores=N_CORES,
    n_chunks=N_CHUNKS,
    debug=False,
):
    nc = bacc.Bacc(
        "TRN2",
        target_bir_lowering=False,
        debug=debug,
        num_devices=n_cores,
    )
    dt = mybir.dt.float16
    d_global = d_local * n_cores

    a_t = nc.dram_tensor("a_t", [k_prev, b], dt, kind="ExternalInput")
    w_prev = nc.dram_tensor("w_prev", [k_prev, d_local], dt, kind="ExternalInput")
    w_up = nc.dram_tensor("w_up", [d_global, f_local], dt, kind="ExternalInput")
    out = nc.dram_tensor("out", [b, f_local], dt, kind="ExternalOutput")

    chunk = b // n_chunks
    groups = [list(range(n_cores))]

    with tile.TileContext(nc) as tc:
        with tc.tile_pool(name="dram", bufs=1, space="DRAM") as dram:
            ag_in = [
                dram.tile([d_local, chunk], dt, name=f"ag_in{c}")
                for c in range(n_chunks)
            ]
            ag_out = [
                dram.tile(
                    [d_global, chunk], dt, name=f"ag_out{c}", addr_space="Shared"
                )
                for c in range(n_chunks)
            ]
            for c in range(n_chunks):
                cs = slice(c * chunk, (c + 1) * chunk)
                # act_T[:, chunk c] = W_prev_shard^T @ A^T[:, chunk c]
                matmul_tile_kernel(
                    tc,
                    w_prev.ap(),
                    a_t.ap()[:, cs],
                    ag_in[c][:],
                )
                nc.gpsimd.collective_compute(
                    "AllGather",
                    mybir.AluOpType.bypass,
                    replica_groups=groups,
                    ins=[ag_in[c].opt()],
                    outs=[ag_out[c].opt()],
                )
            for c in range(n_chunks):
                cs = slice(c * chunk, (c + 1) * chunk)
                # out[chunk c, :] = act_T_global[:, chunk c]^T @ W_up_shard
                matmul_tile_kernel(
                    tc,
                    ag_out[c][:],
                    w_up.ap(),
                    out.ap()[cs, :],
                )
    nc.compile()
    return nc


_NC_CACHE = {}


def _get_nc():
    if "nc" not in _NC_CACHE:
        _NC_CACHE["nc"] = build_nc()
    return _NC_CACHE["nc"]


def run(A_prev, W_prev, W_up, **spmd_kwargs):
    A_t = np.ascontiguousarray(A_prev.T)
    in_maps = []
    for r in range(N_CORES):
        in_maps.append(
            {
                "a_t": A_t,
                "w_prev": np.ascontiguousarray(
                    W_prev[:, r * D_LOCAL : (r + 1) * D_LOCAL]
                ),
                "w_up": np.ascontiguousarray(W_up[:, r * F_LOCAL : (r + 1) * F_LOCAL]),
            }
        )
    nc = _get_nc()
    res = run_bass_kernel_spmd(
        nc, in_maps, core_ids=list(range(N_CORES)), **spmd_kwargs
    )
    out = np.concatenate([res.results[r]["out"] for r in range(N_CORES)], axis=1)
    return out, res


def kernel(A_prev, W_prev, W_up):
    return run(A_prev, W_prev, W_up)[0]
